# revision 8
# baseline (speedup 1.0000x reference)
"""Trainium2 Bass kernel for a 2-layer DGL-style GCN (mean aggregation).

Reference computation:
    h_N  = segmean(feat[src] -> dst)                 # [N, 128]
    h    = relu(concat([feat, h_N]) @ W0.T)          # [N, 128]
    h_N2 = segmean(h[src] -> dst)
    out  = concat([h, h_N2]) @ W1.T                  # [N, 64]

Distribution: dst-range sharding over 8 cores (node n owned by core
n // NPC).  Each core aggregates its own nodes exactly; one AllGather
shares z2 = h @ W1b.T between the passes (segmean(h) @ W1b.T ==
segmean(h @ W1b.T), so the gather moves 64-dim rows).

Per-core layout: 25 groups of 512 dst nodes.  Edges are bucketed by
(group, source-chunk of 25600 rows, dst sub-group of 128); per-sub
capacity is the max edge count over the 8 cores rounded up to 128
(uniform across cores so one program serves all).  One fat dma_gather
per (group, chunk) pulls all its ~2.5k source rows (bf16, 256B rows);
the segment-sum runs as 128-slot x 128-dst selection matmuls in bf16
(1 cycle/row vs 4 for fp32), with the 1/indeg mean weight folded into
the selection matrix.  Aggregates accumulate transposed in PSUM so the
dense layers consume them directly.  Edge metadata (gather indices,
dst-slot ids, weights) is resident in SBUF and shared by both passes,
which use the identical edge order.  The z2 table is bf16 padded to
256-byte rows (dma_gather granularity); its upper half is never read.
All 8 cores run one identical program on different data.
"""

import sys

sys.path.insert(0, "/opt/trn_rl_repo")

from contextlib import ExitStack

import ml_dtypes
import numpy as np

import concourse.bass as bass
import concourse.tile as tile
from concourse import bacc, mybir
from concourse.bass_utils import run_bass_kernel_spmd

F32 = mybir.dt.float32
BF16 = mybir.dt.bfloat16
I16 = mybir.dt.int16
BF = ml_dtypes.bfloat16
P = 128
GRP = 512  # dst nodes per group
SUB = 128  # dst nodes per selection-matmul window
NSUB = GRP // SUB
DL_PAD = 200.0  # dst-slot sentinel for padding slots (!= any iota value)


def _split_sync_waits(nc, max_waits=1):
    """This walrus's codegen rejects instructions carrying more than
    `max_waits` semaphore waits. Hoist the excess onto same-engine nops
    inserted immediately before the offending instruction."""
    import bass_rust

    ctr = 0
    for bb in nc.main_func.blocks:
        insts = bb.instructions
        need = any(
            ins.sync_info is not None and len(ins.sync_info.on_wait) > max_waits
            for ins in insts
        )
        if not need:
            continue
        out = []
        for ins in insts:
            si = ins.sync_info
            if si is not None and len(si.on_wait) > max_waits:
                waits = list(si.on_wait)
                keep, rest = waits[:max_waits], waits[max_waits:]
                while rest:
                    chunk, rest = rest[:max_waits], rest[max_waits:]
                    ctr += 1
                    nop = bass_rust.InstNoOp(
                        name=f"I-waitsplit-{ctr}", engine=ins.engine
                    )
                    nop.sync_info = mybir.SyncInfo(on_wait=chunk, on_update=[])
                    out.append(nop)
                si.on_wait = keep
            out.append(ins)
        insts.clear()
        insts.extend(out)


class _GcnBacc(bacc.Bacc):
    """Bacc whose finalize also splits multi-wait instructions (this
    walrus rejects >1 sync wait on several instruction templates)."""

    def finalize(self):
        if self._finalized:
            return
        self.compile()
        _split_sync_waits(self)
        bass.Bass.finalize(self)


def make_cfg(N, E, D_IN, D_HID, D_OUT, C=8):
    NPC = -(-N // (C * GRP)) * GRP  # per-core nodes, multiple of 512
    N_PAD = C * NPC
    NCH = 4
    assert N_PAD % NCH == 0 and N_PAD // NCH <= 32768
    return {
        "N": N,
        "N_PAD": N_PAD,
        "D_IN": D_IN,
        "D_HID": D_HID,
        "D_OUT": D_OUT,
        "C": C,
        "NPC": NPC,
        "NCH": NCH,
    }


def _kc_offsets(kc):
    """Per-(group, chunk) block/idx column offsets from the kc table.

    kc: [NG][NCH][NSUB] nested tuples of per-sub block counts.
    Returns (blk_off, iw_off, nblocks, total_blocks)."""
    NG, NCH = len(kc), len(kc[0])
    blk_off = [[0] * NCH for _ in range(NG)]
    iw_off = [[0] * NCH for _ in range(NG)]
    nblocks = [[0] * NCH for _ in range(NG)]
    b = 0
    for g in range(NG):
        for c in range(NCH):
            blk_off[g][c] = b
            iw_off[g][c] = b * (SUB // 16)
            nblocks[g][c] = sum(kc[g][c])
            b += nblocks[g][c]
    return blk_off, iw_off, nblocks, b


def build_program(cfg, kc, reps=1, no_cc=False):
    """Emit the per-core SPMD program (identical across cores).

    kc[g][c][s]: number of 128-slot blocks for (group g, source chunk c,
    dst sub-group s) -- the max over cores, so one program fits all.
    reps>1 repeats the whole computation (for slope timing); no_cc
    replaces the AllGather with a local copy so the single-core
    TimelineSim can profile the program."""
    N_PAD, D_IN, D_HID, D_OUT = cfg["N_PAD"], cfg["D_IN"], cfg["D_HID"], cfg["D_OUT"]
    C, NPC, NCH = cfg["C"], cfg["NPC"], cfg["NCH"]
    CH = N_PAD // NCH
    NG = NPC // GRP
    blk_off, iw_off, nblocks, B_TOT = _kc_offsets(kc)
    IW_TOT = B_TOT * (SUB // 16)
    BMAX = max(max(row) for row in nblocks)

    nc = _GcnBacc(None)
    feat_t = nc.declare_dram_parameter("feat16", [N_PAD, D_IN], BF16, isOutput=False)
    featT_t = nc.declare_dram_parameter("featT16", [D_IN, NPC], BF16, isOutput=False)
    idx_t = nc.declare_dram_parameter("idx16", [P, IW_TOT], I16, isOutput=False)
    dl_t = nc.declare_dram_parameter("dl16", [P, B_TOT], F32, isOutput=False)
    wg_t = nc.declare_dram_parameter("wg16", [P, B_TOT], F32, isOutput=False)
    w0a_t = nc.declare_dram_parameter("w0at", [D_IN, D_HID], BF16, isOutput=False)
    w0b_t = nc.declare_dram_parameter("w0bt", [D_IN, D_HID], BF16, isOutput=False)
    w1a_t = nc.declare_dram_parameter("w1at", [D_HID, D_OUT], BF16, isOutput=False)
    w1b_t = nc.declare_dram_parameter("w1bt", [D_HID, D_OUT], BF16, isOutput=False)
    iota_t = nc.declare_dram_parameter("iota", [P, SUB], BF16, isOutput=False)
    ident_t = nc.declare_dram_parameter("ident", [D_OUT, D_OUT], F32, isOutput=False)
    outT_t = nc.declare_dram_parameter("outT", [D_OUT, NPC], F32, isOutput=True)

    with ExitStack() as ctx:
        tc = ctx.enter_context(tile.TileContext(nc))

        const = ctx.enter_context(tc.tile_pool(name="const", bufs=1))
        dram = ctx.enter_context(tc.tile_pool(name="dram", bufs=1, space="DRAM"))
        z2_piece = dram.tile([NPC, P], BF16)
        z2_full = dram.tile([N_PAD, P], BF16)

        w0a_s = const.tile([D_IN, D_HID], BF16, tag="w0a")
        w0b_s = const.tile([D_IN, D_HID], BF16, tag="w0b")
        w1a_s = const.tile([D_HID, D_OUT], BF16, tag="w1a")
        w1b_s = const.tile([D_HID, D_OUT], BF16, tag="w1b")
        iota_s = const.tile([P, SUB], BF16, tag="iota")
        ident_s = const.tile([D_OUT, D_OUT], F32, tag="ident")
        featT_s = const.tile([D_IN, NPC], BF16, tag="featT")
        hT_s = const.tile([D_HID, NPC], BF16, tag="hT")
        idx_s = const.tile([P, IW_TOT], I16, tag="idx")
        dl_s = const.tile([P, B_TOT], F32, tag="dl")
        wg_s = const.tile([P, B_TOT], F32, tag="wg")
        # persistent z2-row staging tiles: odd 64-col halves stay zero forever
        z2r_tiles = [
            const.tile([P, GRP], BF16, tag=f"z2r{i}", name=f"z2r{i}") for i in range(2)
        ]
        for t in z2r_tiles:
            nc.vector.memset(t[:], 0)
        for dst, src in [
            (w0a_s, w0a_t),
            (w0b_s, w0b_t),
            (w1a_s, w1a_t),
            (w1b_s, w1b_t),
            (iota_s, iota_t),
            (ident_s, ident_t),
            (featT_s, featT_t),
            (idx_s, idx_t),
            (dl_s, dl_t),
            (wg_s, wg_t),
        ]:
            nc.sync.dma_start(out=dst[:], in_=src[:])

        gp = ctx.enter_context(tc.tile_pool(name="gp", bufs=2))
        mp = ctx.enter_context(tc.tile_pool(name="mp", bufs=4))
        misc = ctx.enter_context(tc.tile_pool(name="misc", bufs=3))
        pa_p = ctx.enter_context(tc.tile_pool(name="pa", bufs=2, space="PSUM"))
        ph_p = ctx.enter_context(tc.tile_pool(name="ph", bufs=2, space="PSUM"))
        pz_p = ctx.enter_context(tc.tile_pool(name="pz", bufs=2, space="PSUM"))
        pzr_p = ctx.enter_context(tc.tile_pool(name="pzr", bufs=2, space="PSUM"))

        # Hardware rejects dma_gather with more than 1024 indices (verified
        # empirically: 1024 passes, 1280 hangs the exec unit), so each
        # (group, chunk) gather is issued in <=1024-index pieces.
        GMAX = 1024

        def gather_blocks(g, c, table_ap, gtag):
            """Gather all slots of (group g, chunk c) in <=1024-idx pieces."""
            nb = nblocks[g][c]
            cap = nb * SUB
            gt = gp.tile([P, BMAX * SUB], BF16, tag=gtag)
            for s0 in range(0, cap, GMAX):
                n = min(GMAX, cap - s0)
                nc.gpsimd.dma_gather(
                    out_ap=gt[:, s0 : s0 + n].rearrange("p (k e) -> p k e", e=SUB),
                    in_ap=table_ap[c * CH : (c + 1) * CH, :],
                    idxs_ap=idx_s[
                        :, iw_off[g][c] + s0 // 16 : iw_off[g][c] + (s0 + n) // 16
                    ],
                    num_idxs=n,
                    num_idxs_reg=n,
                    elem_size=SUB,
                )
            return gt

        def make_m(g, c, b, mtag):
            m = mp.tile([P, SUB], BF16, tag=mtag)
            col = blk_off[g][c] + b
            nc.any.tensor_scalar(
                out=m[:],
                in0=iota_s[:],
                scalar1=dl_s[:, col : col + 1],
                scalar2=wg_s[:, col : col + 1],
                op0=mybir.AluOpType.is_equal,
                op1=mybir.AluOpType.mult,
            )
            return m

        for _rep in range(reps):
            # ---------------- pass 1 ----------------
            for g in range(NG):
                sl = slice(g * GRP, (g + 1) * GRP)
                pa = pa_p.tile([D_IN, GRP], F32, tag="pa")
                gts = [gather_blocks(g, c, feat_t, f"g1c{c}") for c in range(NCH)]
                # sweep windows sub-major so each psum window sees one
                # contiguous start..stop accumulation run (interleaving
                # starts of different windows corrupts the bank)
                for s in range(NSUB):
                    for c in range(NCH):
                        b0 = sum(kc[g][c][:s])
                        for k in range(kc[g][c][s]):
                            m = make_m(g, c, b0 + k, "m1")
                            nc.tensor.matmul(
                                pa[:, s * SUB : (s + 1) * SUB],
                                lhsT=gts[c][:, (b0 + k) * SUB : (b0 + k + 1) * SUB],
                                rhs=m[:],
                                start=(c == 0 and k == 0),
                                stop=(c == NCH - 1 and k == kc[g][c][s] - 1),
                            )

                aggs = misc.tile([D_IN, GRP], BF16, tag="aggs")
                nc.any.tensor_copy(out=aggs[:], in_=pa[:])
                ph = ph_p.tile([D_HID, GRP], F32, tag="ph")
                nc.tensor.matmul(
                    ph[:], lhsT=w0a_s[:], rhs=featT_s[:, sl], start=True, stop=False
                )
                nc.tensor.matmul(ph[:], lhsT=w0b_s[:], rhs=aggs[:], start=False, stop=True)
                nc.scalar.activation(
                    out=hT_s[:, sl], in_=ph[:], func=mybir.ActivationFunctionType.Relu
                )

                pz = pz_p.tile([D_OUT, GRP], F32, tag="pz")
                nc.tensor.matmul(pz[:], lhsT=w1b_s[:], rhs=hT_s[:, sl], start=True, stop=True)
                z2T = misc.tile([D_OUT, GRP], F32, tag="z2T")
                nc.any.tensor_copy(out=z2T[:], in_=pz[:])
                z2r = z2r_tiles[g % 2]
                for w in range(GRP // P):
                    pzr = pzr_p.tile([P, D_OUT], F32, tag="pzr")
                    nc.tensor.transpose(
                        out=pzr[:], in_=z2T[:, w * P : (w + 1) * P], identity=ident_s[:]
                    )
                    nc.any.tensor_copy(out=z2r[:, w * P : w * P + D_OUT], in_=pzr[:])
                nc.sync.dma_start(
                    out=z2_piece[sl, :].rearrange("(w p) e -> p w e", p=P),
                    in_=z2r[:].rearrange("p (w e) -> p w e", e=P),
                )

            # ---------------- all-gather z2 ----------------
            if no_cc:
                nc.sync.dma_start(out=z2_full[0:NPC, :], in_=z2_piece[:])
            else:
                nc.gpsimd.collective_compute(
                    "AllGather",
                    mybir.AluOpType.bypass,
                    replica_groups=[list(range(C))],
                    ins=[z2_piece[:]],
                    outs=[z2_full[:]],
                )

            # ---------------- pass 2 ----------------
            for g in range(NG):
                sl = slice(g * GRP, (g + 1) * GRP)
                po = pz_p.tile([D_OUT, GRP], F32, tag="pz")
                nc.tensor.matmul(
                    po[:], lhsT=w1a_s[:], rhs=hT_s[:, sl], start=True, stop=False
                )
                gt2s = [gather_blocks(g, c, z2_full, f"g2c{c}") for c in range(NCH)]
                for s in range(NSUB):
                    for c in range(NCH):
                        b0 = sum(kc[g][c][:s])
                        for k in range(kc[g][c][s]):
                            m = make_m(g, c, b0 + k, "m2")
                            nc.tensor.matmul(
                                po[:, s * SUB : (s + 1) * SUB],
                                lhsT=gt2s[c][:, (b0 + k) * SUB : (b0 + k) * SUB + D_OUT],
                                rhs=m[:],
                                start=False,
                                stop=(c == NCH - 1 and k == kc[g][c][s] - 1),
                            )
                o_s = misc.tile([D_OUT, GRP], F32, tag="os")
                nc.any.tensor_copy(out=o_s[:], in_=po[:])
                nc.sync.dma_start(out=outT_t[:, sl], in_=o_s[:])

    return nc


def prep_inputs(feat, edge_src, edge_dst, W0, W1, cfg):
    """Host-side index/layout prep. Returns (per-core input maps, kc)."""
    N, D_IN = feat.shape
    N_PAD, D_HID, D_OUT = cfg["N_PAD"], cfg["D_HID"], cfg["D_OUT"]
    C, NPC, NCH = cfg["C"], cfg["NPC"], cfg["NCH"]
    CH = N_PAD // NCH
    NG = NPC // GRP
    E = edge_src.shape[0]

    indeg = np.bincount(edge_dst, minlength=N).astype(np.float32)
    ew = (1.0 / np.maximum(indeg, 1.0))[edge_dst].astype(np.float32)

    core = edge_dst // NPC
    g = (edge_dst % NPC) // GRP
    s = (edge_dst % GRP) // SUB
    c = edge_src // CH
    n_subs = C * NG * NCH * NSUB
    sub_id = ((core * NG + g) * NCH + c) * NSUB + s

    cnt = np.bincount(sub_id, minlength=n_subs).reshape(C, NG, NCH, NSUB)
    kc_arr = np.maximum(1, -(-cnt.max(axis=0) // SUB))  # [NG, NCH, NSUB]
    caps = kc_arr * SUB
    cap_flat = caps.reshape(-1)
    off_flat = np.zeros(NG * NCH * NSUB, dtype=np.int64)
    off_flat[1:] = np.cumsum(cap_flat)[:-1]
    total = int(cap_flat.sum())

    order = np.argsort(sub_id, kind="stable")
    src_o = edge_src[order]
    dst_o = edge_dst[order]
    ew_o = ew[order]
    sid_o = sub_id[order]
    core_o = core[order]

    counts_all = np.bincount(sub_id, minlength=n_subs)
    starts_all = np.zeros(n_subs, dtype=np.int64)
    starts_all[1:] = np.cumsum(counts_all)[:-1]
    slot_in_sub = np.arange(E, dtype=np.int64) - starts_all[sid_o]
    pos = off_flat[sid_o % (NG * NCH * NSUB)] + slot_in_sub

    idx_arr = np.zeros((C, total), dtype=np.int16)
    dl_arr = np.full((C, total), DL_PAD, dtype=np.float32)
    wg_arr = np.zeros((C, total), dtype=np.float32)
    idx_arr[core_o, pos] = (src_o % CH).astype(np.int16)
    dl_arr[core_o, pos] = (dst_o % SUB).astype(np.float32)
    wg_arr[core_o, pos] = ew_o

    # device layouts --------------------------------------------------
    # per (g, c) segment: idx slot i -> partition i % 16, col i // 16,
    # replicated 8x; dl/wg slot i -> partition i % 128, col i // 128.
    idx_parts, dl_parts, wg_parts = [], [], []
    caps_gc = caps.sum(axis=2)  # [NG, NCH]
    off_gc = off_flat.reshape(NG, NCH, NSUB)[:, :, 0]
    for gg in range(NG):
        for cc in range(NCH):
            cap = int(caps_gc[gg, cc])
            o = int(off_gc[gg, cc])
            seg = idx_arr[:, o : o + cap].reshape(C, cap // 16, 16)
            idx_parts.append(np.tile(seg.transpose(0, 2, 1), (1, 8, 1)))
            dseg = dl_arr[:, o : o + cap].reshape(C, cap // SUB, SUB)
            dl_parts.append(dseg.transpose(0, 2, 1))
            wseg = wg_arr[:, o : o + cap].reshape(C, cap // SUB, SUB)
            wg_parts.append(wseg.transpose(0, 2, 1))
    idx_dev = np.ascontiguousarray(np.concatenate(idx_parts, axis=2))
    dl_dev = np.ascontiguousarray(np.concatenate(dl_parts, axis=2))
    wg_dev = np.ascontiguousarray(np.concatenate(wg_parts, axis=2))

    feat_pad = np.zeros((N_PAD, D_IN), dtype=BF)
    feat_pad[:N] = feat.astype(BF)
    featT = np.zeros((D_IN, N_PAD), dtype=BF)
    featT[:, :N] = feat.T.astype(BF)

    w0a = np.ascontiguousarray(W0[:, :D_IN].T).astype(BF)
    w0b = np.ascontiguousarray(W0[:, D_IN:].T).astype(BF)
    w1a = np.ascontiguousarray(W1[:, :D_HID].T).astype(BF)
    w1b = np.ascontiguousarray(W1[:, D_HID:].T).astype(BF)
    iota = np.tile(np.arange(SUB, dtype=np.float32), (P, 1)).astype(BF)
    ident = np.eye(D_OUT, dtype=np.float32)

    in_maps = []
    for cc in range(C):
        in_maps.append(
            {
                "feat16": feat_pad,
                "featT16": np.ascontiguousarray(featT[:, cc * NPC : (cc + 1) * NPC]),
                "idx16": idx_dev[cc],
                "dl16": dl_dev[cc],
                "wg16": wg_dev[cc],
                "w0at": w0a,
                "w0bt": w0b,
                "w1at": w1a,
                "w1bt": w1b,
                "iota": iota,
                "ident": ident,
            }
        )
    kc = tuple(
        tuple(tuple(int(x) for x in kc_arr[gg, cc]) for cc in range(NCH))
        for gg in range(NG)
    )
    return in_maps, kc


_PROGRAM_CACHE = {}


def _get_program(cfg, kc, reps):
    key = (tuple(sorted(cfg.items())), kc, reps)
    if key not in _PROGRAM_CACHE:
        nc = build_program(cfg, kc, reps=reps)
        nc.finalize()
        _PROGRAM_CACHE[key] = nc
    return _PROGRAM_CACHE[key]


def _run(feat, edge_src, edge_dst, W0, W1, C=8, trace=False):
    N, D_IN = feat.shape
    cfg = make_cfg(N, edge_src.shape[0], D_IN, W0.shape[0], W1.shape[0], C)
    in_maps, kc = prep_inputs(feat, edge_src, edge_dst, W0, W1, cfg)
    nc = _get_program(cfg, kc, 1)

    res = run_bass_kernel_spmd(nc, in_maps, core_ids=list(range(C)), trace=trace)
    pieces = [res.results[c]["outT"].T for c in range(C)]  # [NPC, D_OUT]
    out = np.concatenate(pieces, axis=0)[:N]
    return np.ascontiguousarray(out.astype(np.float32)), res


def bench(feat, edge_src, edge_dst, W0, W1, C=8, iters=10, reps=1):
    """Time device execution of the compiled program: inputs pre-staged on
    device, jit without donation, min over `iters` calls."""
    import time

    import jax
    from jax.sharding import Mesh, NamedSharding, PartitionSpec

    try:
        from jax.experimental.shard_map import shard_map
    except ImportError:
        from jax.shard_map import shard_map
    from concourse import bass2jax
    from concourse.bass2jax import _bass_exec_p

    feat = np.asarray(feat, dtype=np.float32)
    edge_src = np.asarray(edge_src, dtype=np.int32)
    edge_dst = np.asarray(edge_dst, dtype=np.int32)
    W0 = np.asarray(W0, dtype=np.float32)
    W1 = np.asarray(W1, dtype=np.float32)
    cfg = make_cfg(feat.shape[0], edge_src.shape[0], feat.shape[1], W0.shape[0], W1.shape[0], C)
    in_maps, kc = prep_inputs(feat, edge_src, edge_dst, W0, W1, cfg)
    nc = _get_program(cfg, kc, reps)

    bass2jax.install_neuronx_cc_hook()
    import concourse.mybir as mb

    part_name = nc.partition_id_tensor.name if nc.partition_id_tensor else None
    in_names, out_names, out_avals, zero_outs = [], [], [], []
    for alloc in nc.m.functions[0].allocations:
        if not isinstance(alloc, mb.MemoryLocationSet):
            continue
        name = alloc.memorylocations[0].name
        if alloc.kind == "ExternalInput":
            if name != part_name:
                in_names.append(name)
        elif alloc.kind == "ExternalOutput":
            shape = tuple(alloc.tensor_shape)
            dtype = mb.dt.np(alloc.dtype)
            out_names.append(name)
            out_avals.append(jax.core.ShapedArray(shape, dtype))
            zero_outs.append(np.zeros(shape, dtype))
    n_params = len(in_names)
    all_in_names = in_names + out_names
    if part_name is not None:
        all_in_names.append(part_name)

    def _body(*args):
        operands = list(args)
        if part_name is not None:
            operands.append(bass2jax.partition_id_tensor())
        return tuple(
            _bass_exec_p.bind(
                *operands,
                out_avals=tuple(out_avals),
                in_names=tuple(all_in_names),
                out_names=tuple(out_names),
                lowering_input_output_aliases=(),
                sim_require_finite=True,
                sim_require_nnan=True,
                nc=nc,
            )
        )

    devices = jax.devices()[:C]
    mesh = Mesh(np.asarray(devices), ("core",))
    spec = PartitionSpec("core")
    n_args = n_params + len(out_names)
    fn = jax.jit(
        shard_map(
            _body,
            mesh=mesh,
            in_specs=(spec,) * n_args,
            out_specs=(spec,) * len(out_names),
            check_rep=False,
        )
    )
    concat_in = [
        np.concatenate([np.asarray(in_maps[c][nm]) for c in range(C)], axis=0)
        for nm in in_names
    ] + [np.zeros((C * z.shape[0], *z.shape[1:]), z.dtype) for z in zero_outs]
    sharding = NamedSharding(mesh, spec)
    dev_in = [jax.device_put(a, sharding) for a in concat_in]
    # warmup (compiles + first exec)
    r = fn(*dev_in)
    jax.block_until_ready(r)
    times = []
    for _ in range(iters):
        t0 = time.perf_counter()
        r = fn(*dev_in)
        jax.block_until_ready(r)
        times.append(time.perf_counter() - t0)
    return {
        "min_s": min(times),
        "median_s": sorted(times)[len(times) // 2],
        "all_s": times,
        "out": np.asarray(r[0]),
        "out_names": out_names,
        "cfg": cfg,
    }


def kernel(feat, edge_src, edge_dst, W0, W1):
    out, _ = _run(
        np.asarray(feat, dtype=np.float32),
        np.asarray(edge_src, dtype=np.int32),
        np.asarray(edge_dst, dtype=np.int32),
        np.asarray(W0, dtype=np.float32),
        np.asarray(W1, dtype=np.float32),
    )
    return out
